# revision 20
# baseline (speedup 1.0000x reference)
"""GAT classifier on 8 TRN2 NeuronCores (graph/data parallel).

- Nodes dealt across 8 cores; all dense per-node math runs feature-major on
  PE/ACT, sharded by node. Batchnorm stats via tiny AllReduce.
- Per GAT layer each core writes its shard of an augmented gather table
  row[v] = [h[v] bf16 | a_s[v] f32-bits | pad] (256B/512B rows) which is
  AllGathered. Edges live in padded-CSR rectangles (128 dst nodes x D source
  slots, split into two int16-indexed table halves); rows are fetched with
  the Q7 dma_gather ucode; segment softmax is computed per-partition; the
  weighted message sum is an in-place vector multiply + reduce.
"""

import numpy as np
import ml_dtypes

import concourse.bacc as bacc
import concourse.bass as bass  # noqa: F401
import concourse.mybir as mybir
import concourse.tile as tile
import concourse.bass_interp as bass_interp
from concourse.masks import make_identity

F32 = mybir.dt.float32
BF16 = mybir.dt.bfloat16
I16 = mybir.dt.int16
AX = mybir.AxisListType
OP = mybir.AluOpType
AF = mybir.ActivationFunctionType

N_NODES = 50000
F_IN = 32
F_ENC = 128
MID = (64, 128, 64)
EPS = 1e-5
NEG_SLOPE = 0.2

NCORES = 8
NPC = N_NODES // NCORES          # 6250
NGRP = (NPC + 127) // 128        # 49
FMW = NGRP * 128                 # 6272
NSH = FMW + 1                    # 6273 shard rows (last = sentinel)
HALF = 4 * NSH                   # 25092
NT = 8 * NSH                     # 50184
SENT = NSH - 1                   # 6272
NCHUNK = 512
CC_BYTES = 32768                 # gather chunk bytes per partition

_CACHE = {}

import itertools

_uid = itertools.count()


# ---------------------------------------------------------------- host prep
def _build_chunks(d_lo, d_hi, cc_slots):
    chunks, g = [], 0
    while g < NGRP:
        g0, slots = g, 0
        while g < NGRP and (slots + d_lo[g] + d_hi[g] <= cc_slots or g == g0):
            slots += d_lo[g] + d_hi[g]
            g += 1
        gl = list(range(g0, g))
        chunks.append(dict(
            groups=gl,
            slots_lo=int(sum(d_lo[x] for x in gl)),
            slots_hi=int(sum(d_hi[x] for x in gl)),
        ))
    return chunks


def _host_prep(edge_index):
    src = np.asarray(edge_index[0], dtype=np.int64)
    dst = np.asarray(edge_index[1], dtype=np.int64)
    deg = np.bincount(dst, minlength=N_NODES) + 1

    by_deg = np.argsort(-deg, kind="stable")
    half_bit = np.empty(N_NODES, np.int8)
    half_bit[by_deg] = (np.arange(N_NODES) % 2).astype(np.int8)

    src_half = half_bit[src]
    lo = np.bincount(dst[src_half == 0], minlength=N_NODES) + (half_bit == 0)
    hi = np.bincount(dst[src_half == 1], minlength=N_NODES) + (half_bit == 1)

    node_of = np.empty((NCORES, NPC), np.int64)
    for h in range(2):
        nodes_h = np.where(half_bit == h)[0]
        order = nodes_h[np.lexsort((hi[nodes_h], lo[nodes_h]))]
        for c in range(4):
            node_of[4 * h + c] = order[c::4]

    row_of = np.empty(N_NODES, np.int64)
    slot_of = np.empty(N_NODES, np.int64)
    core_of = np.empty(N_NODES, np.int64)
    for c in range(NCORES):
        row_of[node_of[c]] = NSH * c + np.arange(NPC)
        slot_of[node_of[c]] = np.arange(NPC)
        core_of[node_of[c]] = c

    lo_cs, hi_cs = lo[node_of], hi[node_of]
    d_lo = np.zeros(NGRP, np.int64)
    d_hi = np.zeros(NGRP, np.int64)
    for g in range(NGRP):
        s0, s1 = 128 * g, min(128 * g + 128, NPC)
        d_lo[g] = lo_cs[:, s0:s1].max()
        d_hi[g] = hi_cs[:, s0:s1].max()
    pad_ratio = float(128 * (d_lo.sum() + d_hi.sum())
                      ) / (len(src) / NCORES + NPC)

    # two chunk structures: one per row width
    structs = {}
    for w16, nameidx in ((128, 0), (256, 1)):
        cc = CC_BYTES // (2 * w16)
        chs = _build_chunks(d_lo, d_hi, cc)
        mx = max(c["slots_lo"] + c["slots_hi"] for c in chs)
        structs[w16] = dict(cc=cc, chunks=chs, max_stot=mx)

    # per-core padded adjacency matrices (slot-major)
    src_row_all = row_of[src]
    dmaxlo, dmaxhi = int(d_lo.max()), int(d_hi.max())
    pad_lo = np.full((NCORES, NPC, dmaxlo), SENT, np.int64)
    pad_hi = np.full((NCORES, NPC, dmaxhi), SENT, np.int64)
    for c in range(NCORES):
        mask = core_of[dst] == c
        slots = slot_of[dst[mask]]
        rows = src_row_all[mask]
        # self loops
        slots = np.concatenate([slots, np.arange(NPC)])
        rows = np.concatenate([rows, NSH * c + np.arange(NPC)])
        for which, padm in ((0, pad_lo), (1, pad_hi)):
            sel = (rows < HALF) if which == 0 else (rows >= HALF)
            s_sl, s_rw = slots[sel], rows[sel]
            o = np.argsort(s_sl, kind="stable")
            s_sl, s_rw = s_sl[o], s_rw[o]
            st = np.searchsorted(s_sl, np.arange(NPC))
            pos = np.arange(len(s_sl)) - st[s_sl]
            padm[c, s_sl, pos] = s_rw - (0 if which == 0 else HALF)

    # packed int16 index tensors, one per structure
    idxt = {}
    for w16, s in structs.items():
        totc = 8 * sum(c["slots_lo"] + c["slots_hi"] for c in s["chunks"])
        arr = np.empty((NCORES, 128, totc), np.int16)
        for c in range(NCORES):
            cols = []
            for ch in s["chunks"]:
                for which, padm, dsel in ((0, pad_lo, d_lo), (1, pad_hi, d_hi)):
                    for g in ch["groups"]:
                        s0 = 128 * g
                        blk = np.full((128, dsel[g]), SENT, np.int64)
                        real = min(128, NPC - s0)
                        blk[:real] = padm[c, s0:s0 + real, :dsel[g]]
                        cols.append(blk.T)          # [d, 128] col-major
            flat = np.concatenate(cols, 0).reshape(-1)
            wrapped = flat.reshape(-1, 16).T
            arr[c] = np.tile(wrapped, (8, 1)).astype(np.int16)
        s["totc"] = totc
        idxt[w16] = arr

    return dict(node_of=node_of, d_lo=d_lo, d_hi=d_hi, structs=structs,
                idxt=idxt, pad_ratio=pad_ratio)


def _pack_weights(enc, gat, bn, nup, out_mlp):
    t = {}

    def addW(name, W, b=None):
        W = np.asarray(W, np.float32)
        K, M = W.shape
        for ki in range((K + 127) // 128):
            for mi in range((M + 127) // 128):
                t[f"{name}_k{ki}m{mi}"] = np.ascontiguousarray(
                    W[128 * ki:128 * ki + 128, 128 * mi:128 * mi + 128])
        if b is not None:
            b = np.asarray(b, np.float32)
            for mi in range((M + 127) // 128):
                t[f"{name}_b{mi}"] = np.ascontiguousarray(
                    b[128 * mi:128 * mi + 128].reshape(-1, 1))

    for i, (W, b) in enumerate(enc):
        addW(f"enc{i}", W, b)
    for l, p in enumerate(gat):
        addW(f"gatW{l}", p["W"])
        fo = np.asarray(p["W"]).shape[1]
        t[f"asrc{l}"] = np.tile(np.asarray(p["a_src"], np.float32).reshape(1, fo), (128, 1))
        t[f"adst{l}"] = np.tile(np.asarray(p["a_dst"], np.float32).reshape(1, fo), (128, 1))
    for l, p in enumerate(bn):
        t[f"gamma{l}"] = np.asarray(p["gamma"], np.float32).reshape(-1, 1)
        t[f"beta{l}"] = np.asarray(p["beta"], np.float32).reshape(-1, 1)
    for s, layers in enumerate(nup):
        for i, (W, b) in enumerate(layers):
            addW(f"nup{s}_{i}", W, b)
    for i, (W, b) in enumerate(out_mlp):
        addW(f"out{i}", W, b)
    t["out3n_k0m0"] = t["out3_k0m0"]
    t["out3n_b0"] = -t["out3_b0"]

    for fo, name in ((64, "sent64"), (128, "sent128")):
        row = np.zeros(2 * fo, np.uint16)
        bits = int(np.float32(-1e30).view(np.uint32))
        row[fo] = bits & 0xFFFF
        row[fo + 1] = bits >> 16
        t[name] = row.view(ml_dtypes.bfloat16).reshape(1, 2 * fo)
    return t


# ---------------------------------------------------------------- device
def _build(meta, wshapes):
    import os
    KSTAGE = int(os.environ.get("KSTAGE", "99"))
    nc = bacc.Bacc("TRN2", target_bir_lowering=False, debug=False,
                   num_devices=NCORES)

    x_in = nc.dram_tensor("x_in", [F_IN, NPC], F32, kind="ExternalInput").ap()
    idx_in = {
        w16: nc.dram_tensor(f"idx{w16}", [128, meta["structs"][w16]["totc"]],
                            I16, kind="ExternalInput").ap()
        for w16 in (128, 256)
    }
    out_ext = nc.dram_tensor("out", [1, NPC], F32, kind="ExternalOutput").ap()

    win = {}
    for name, shp in wshapes.items():
        dt = BF16 if name.startswith("sent") else F32
        win[name] = nc.dram_tensor(name, list(shp), dt, kind="ExternalInput").ap()

    shard, table, bn_in, bn_out = [], [], [], []
    for l in range(3):
        w16 = 2 * MID[l]
        shard.append(nc.dram_tensor(f"shard{l}", [NSH, w16], BF16,
                                    kind="Internal").ap())
        table.append(nc.dram_tensor(f"table{l}", [NT, w16], BF16,
                                    kind="Internal", addr_space="Shared").ap())
        bn_in.append(nc.dram_tensor(f"bnin{l}", [MID[l], 2], F32,
                                    kind="Internal").ap())
        bn_out.append(nc.dram_tensor(f"bnout{l}", [MID[l], 2], F32,
                                     kind="Internal", addr_space="Shared").ap())

    d_lo, d_hi = meta["d_lo"], meta["d_hi"]
    rg = [list(range(NCORES))]

    from contextlib import ExitStack

    with tile.TileContext(nc) as tc, ExitStack() as es:
        cpool = es.enter_context(tc.tile_pool(name="const", bufs=1))
        fmpool = es.enter_context(tc.tile_pool(name="fm", bufs=3))
        mlppool = es.enter_context(tc.tile_pool(name="mlp", bufs=2))
        gpool = es.enter_context(tc.tile_pool(name="gath", bufs=2))
        epool = es.enter_context(tc.tile_pool(name="edge", bufs=3))
        ipool = es.enter_context(tc.tile_pool(name="idx", bufs=2))
        pspool = es.enter_context(tc.tile_pool(name="ps", bufs=4, space="PSUM"))
        pstp = es.enter_context(tc.tile_pool(name="pstp", bufs=4, space="PSUM"))

        ident = cpool.tile([128, 128], F32)
        make_identity(nc, ident[:])

        wsb = {}
        for name, shp in wshapes.items():
            dt = BF16 if name.startswith("sent") else F32
            wt = cpool.tile(list(shp), dt, tag=name)
            nc.sync.dma_start(out=wt[:], in_=win[name][:])
            wsb[name] = wt

        def mlp_chain(names, dims, funcs, src_full, out_tag, scales=None):
            """Streaming multi-layer MLP, feature-major.

            src_full: list of full-width k-block tiles for the input.
            Returns list of full-width m-block tiles of the last layer.
            """
            M_last = dims[-1]
            nm_last = (M_last + 127) // 128
            outs = [fmpool.tile([min(M_last - 128 * mi, 128), FMW], F32,
                                tag="fm", name=f"mlpout_{next(_uid)}") for mi in range(nm_last)]
            for c0 in range(0, FMW, NCHUNK):
                w = min(NCHUNK, FMW - c0)
                cur = [s[:, c0:c0 + w] for s in src_full]
                for li, name in enumerate(names):
                    K, M = dims[li], dims[li + 1]
                    nk, nm = (K + 127) // 128, (M + 127) // 128
                    last = li == len(names) - 1
                    nxt = []
                    for mi in range(nm):
                        mw = min(M - 128 * mi, 128)
                        ps = pspool.tile([mw, NCHUNK], F32)
                        for ki in range(nk):
                            nc.tensor.matmul(
                                ps[:, :w], wsb[f"{name}_k{ki}m{mi}"][:],
                                cur[ki], start=(ki == 0), stop=(ki == nk - 1))
                        if last:
                            ot = outs[mi][:, c0:c0 + w]
                        else:
                            tmp = mlppool.tile([mw, NCHUNK], F32,
                                               tag=f"t{li}_{mi}")
                            ot = tmp[:, :w]
                        nc.scalar.activation(out=ot, in_=ps[:, :w],
                                             func=funcs[li],
                                             scale=(scales[li] if scales else 1.0),
                                             bias=wsb[f"{name}_b{mi}"][:])
                        nxt.append(ot)
                    cur = nxt
            return outs

        # ---------------- encoder ----------------
        x_sb = fmpool.tile([F_IN, FMW], F32, tag="fm")
        nc.vector.memset(x_sb[:], 0.0)
        nc.sync.dma_start(out=x_sb[:, 0:NPC], in_=x_in[:])
        x_cur = mlp_chain(["enc0", "enc1", "enc2", "enc3"],
                          [F_IN, 128, 256, 128, 128],
                          [AF.Relu, AF.Relu, AF.Relu, AF.Identity],
                          [x_sb], "fm")

        # ---------------- GAT layers ----------------
        nlayers = 0 if KSTAGE == 0 else (1 if KSTAGE < 4 else (2 if KSTAGE == 4 else 3))
        for l in range(nlayers):
            fin = F_ENC if l == 0 else MID[l - 1]
            fout = MID[l]
            w16 = 2 * fout
            s = meta["structs"][w16]

            # h = x @ W (full width), block-transpose + table shard writes
            hfm = fmpool.tile([fout, FMW], F32, tag="fm")
            for c0 in range(0, FMW, NCHUNK):
                w = min(NCHUNK, FMW - c0)
                ps = pspool.tile([fout, NCHUNK], F32)
                nc.tensor.matmul(ps[:, :w], wsb[f"gatW{l}_k0m0"][:],
                                 x_cur[0][:, c0:c0 + w], start=True, stop=True)
                nc.scalar.activation(out=hfm[:, c0:c0 + w], in_=ps[:, :w],
                                     func=AF.Identity, bias=0.0)
            a_d_all = epool.tile([128, NGRP], F32, tag="a_d_all")
            for g in range(NGRP):
                tp = pstp.tile([128, fout], F32, tag="tp")
                nc.tensor.transpose(tp[:], hfm[:, 128 * g:128 * g + 128],
                                    ident[0:fout, 0:fout])
                hnm = epool.tile([128, fout], F32, tag="hnm")
                nc.scalar.copy(out=hnm[:], in_=tp[:])
                rowt = epool.tile([128, w16], BF16, tag="rowt")
                nc.vector.memset(rowt[:, fout + 2:], 0.0)
                nc.vector.tensor_copy(out=rowt[:, 0:fout], in_=hnm[:])
                scr = epool.tile([128, fout], F32, tag="ttr_scr")
                nc.vector.tensor_tensor(out=scr[:], in0=hnm[:],
                                        in1=wsb[f"asrc{l}"][:], op=OP.mult)
                nc.vector.tensor_reduce(
                    out=rowt[:, fout:fout + 2].bitcast(F32), in_=scr[:],
                    axis=AX.X, op=OP.add)
                nc.vector.tensor_tensor(out=scr[:], in0=hnm[:],
                                        in1=wsb[f"adst{l}"][:], op=OP.mult)
                nc.vector.tensor_reduce(
                    out=a_d_all[:, g:g + 1], in_=scr[:], axis=AX.X, op=OP.add)
                r0 = 128 * g
                nc.sync.dma_start(out=shard[l][r0:r0 + 128, :], in_=rowt[:])
            sent = wsb["sent64" if fout == 64 else "sent128"]
            nc.sync.dma_start(out=shard[l][NSH - 1:NSH, :], in_=sent[:])
            nc.gpsimd.collective_compute(
                "AllGather", OP.bypass, replica_groups=rg,
                ins=[shard[l][:]], outs=[table[l][:]])

            # ---- edge phase ----
            if KSTAGE == 1:
                break
            y = fmpool.tile([fout, FMW], F32, tag="fm")
            icol = 0
            for ch in s["chunks"]:
                slo, shi = ch["slots_lo"], ch["slots_hi"]
                stot = slo + shi
                idxtile = ipool.tile([128, 8 * s["max_stot"]], I16, tag="idxt")
                nc.sync.dma_start(out=idxtile[:, 0:8 * stot],
                                  in_=idx_in[w16][:, icol:icol + 8 * stot])
                gath = gpool.tile([128, s["max_stot"] * w16], BF16, tag="gath")
                if slo:
                    nc.gpsimd.dma_gather(
                        out_ap=gath[:, 0:slo * w16].rearrange(
                            "p (c w) -> p c w", w=w16),
                        in_ap=table[l][0:HALF], idxs_ap=idxtile[:, 0:8 * slo],
                        num_idxs=128 * slo, num_idxs_reg=128 * slo,
                        elem_size=w16, single_packet=False)
                if shi:
                    nc.gpsimd.dma_gather(
                        out_ap=gath[:, slo * w16:stot * w16].rearrange(
                            "p (c w) -> p c w", w=w16),
                        in_ap=table[l][HALF:NT],
                        idxs_ap=idxtile[:, 8 * slo:8 * stot],
                        num_idxs=128 * shi, num_idxs_reg=128 * shi,
                        elem_size=w16, single_packet=False)
                lo_off, hi_off = 0, slo
                for g in ch["groups"]:
                    dlo, dhi = int(d_lo[g]), int(d_hi[g])
                    D = dlo + dhi

                    def slot3(off, n):
                        return gath[:, off * w16:(off + n) * w16].rearrange(
                            "p (c w) -> p c w", w=w16)

                    def as_view(off, n):
                        return slot3(off, n)[:, :, fout:fout + 2].bitcast(
                            F32).rearrange("p c w -> p (c w)")

                    ev = epool.tile([128, D], F32, tag="ev")
                    nc.vector.tensor_scalar(
                        out=ev[:, 0:dlo], in0=as_view(lo_off, dlo),
                        scalar1=a_d_all[:, g:g + 1], scalar2=None, op0=OP.add)
                    if dhi:
                        nc.vector.tensor_scalar(
                            out=ev[:, dlo:D], in0=as_view(hi_off, dhi),
                            scalar1=a_d_all[:, g:g + 1], scalar2=None,
                            op0=OP.add)
                    lr = epool.tile([128, D], F32, tag="lr")
                    nc.vector.scalar_tensor_tensor(
                        out=lr[:], in0=ev[:], scalar=NEG_SLOPE, in1=ev[:],
                        op0=OP.mult, op1=OP.max)
                    m = epool.tile([128, 1], F32, tag="m")
                    nc.vector.tensor_reduce(out=m[:], in_=lr[:], axis=AX.X,
                                            op=OP.max)
                    negm = epool.tile([128, 1], F32, tag="negm")
                    nc.vector.tensor_scalar_mul(negm[:], m[:], -1.0)
                    ex = epool.tile([128, D], BF16, tag="ex")
                    z = epool.tile([128, 1], F32, tag="z")
                    nc.scalar.activation(out=ex[:], in_=lr[:], func=AF.Exp,
                                         bias=negm[:], accum_out=z[:])
                    rz = epool.tile([128, 1], F32, tag="rz")
                    nc.vector.reciprocal(rz[:], z[:])
                    # in-place weighted messages: h *= ex (a_s columns untouched)
                    nc.vector.tensor_tensor(
                        out=slot3(lo_off, dlo)[:, :, 0:fout],
                        in0=slot3(lo_off, dlo)[:, :, 0:fout],
                        in1=ex[:, 0:dlo].rearrange("p c -> p c ()").to_broadcast(
                            [128, dlo, fout]), op=OP.mult)
                    if dhi:
                        nc.vector.tensor_tensor(
                            out=slot3(hi_off, dhi)[:, :, 0:fout],
                            in0=slot3(hi_off, dhi)[:, :, 0:fout],
                            in1=ex[:, dlo:D].rearrange(
                                "p c -> p c ()").to_broadcast(
                                [128, dhi, fout]), op=OP.mult)
                    og = epool.tile([128, fout], F32, tag="og")
                    oglo = epool.tile([128, fout], F32, tag="oglo")
                    nc.vector.tensor_reduce(
                        out=oglo[:],
                        in_=slot3(lo_off, dlo)[:, :, 0:fout].rearrange(
                            "p c f -> p f c"), axis=AX.X, op=OP.add)
                    if dhi:
                        nc.vector.tensor_reduce(
                            out=og[:],
                            in_=slot3(hi_off, dhi)[:, :, 0:fout].rearrange(
                                "p c f -> p f c"), axis=AX.X, op=OP.add)
                        nc.vector.tensor_tensor(out=og[:], in0=og[:],
                                                in1=oglo[:], op=OP.add)
                    else:
                        og = oglo
                    nc.vector.tensor_scalar_mul(og[:], og[:], rz[:])
                    tp2 = pstp.tile([fout, 128], F32, tag="tp")
                    nc.tensor.transpose(tp2[:], og[:], ident[:])
                    nc.scalar.copy(out=y[:, 128 * g:128 * g + 128], in_=tp2[:])
                    lo_off += dlo
                    hi_off += dhi
                icol += 8 * stot

            # ---- batchnorm + relu ----
            if KSTAGE == 2:
                x_cur = [y]
                break
            s1 = epool.tile([fout, 1], F32, tag="s1")
            nc.vector.tensor_reduce(out=s1[:], in_=y[:, 0:NPC], axis=AX.X,
                                    op=OP.add)
            s2p = epool.tile([fout, 13], F32, tag="s2p")
            for ci, c0 in enumerate(range(0, NPC, NCHUNK)):
                w = min(NCHUNK, NPC - c0)
                sqt = mlppool.tile([fout, NCHUNK], F32, tag="sqt")
                nc.scalar.activation(out=sqt[:, :w], in_=y[:, c0:c0 + w],
                                     func=AF.Square,
                                     accum_out=s2p[:, ci:ci + 1])
            s2 = epool.tile([fout, 1], F32, tag="s2")
            nc.vector.tensor_reduce(out=s2[:], in_=s2p[:], axis=AX.X, op=OP.add)
            st = epool.tile([fout, 2], F32, tag="st")
            nc.vector.tensor_copy(out=st[:, 0:1], in_=s1[:])
            nc.vector.tensor_copy(out=st[:, 1:2], in_=s2[:])
            nc.sync.dma_start(out=bn_in[l][:], in_=st[:])
            nc.gpsimd.collective_compute(
                "AllReduce", OP.add, replica_groups=rg,
                ins=[bn_in[l][:]], outs=[bn_out[l][:]])
            stg = epool.tile([fout, 2], F32, tag="stg")
            nc.sync.dma_start(out=stg[:], in_=bn_out[l][:])
            mu = epool.tile([fout, 1], F32, tag="mu")
            nc.vector.tensor_scalar_mul(mu[:], stg[:, 0:1], 1.0 / N_NODES)
            m2 = epool.tile([fout, 1], F32, tag="m2")
            nc.vector.tensor_scalar_mul(m2[:], stg[:, 1:2], 1.0 / N_NODES)
            musq = epool.tile([fout, 1], F32, tag="musq")
            nc.vector.tensor_tensor(out=musq[:], in0=mu[:], in1=mu[:],
                                    op=OP.mult)
            var = epool.tile([fout, 1], F32, tag="var")
            nc.vector.tensor_tensor(out=var[:], in0=m2[:], in1=musq[:],
                                    op=OP.subtract)
            nc.vector.tensor_scalar_add(var[:], var[:], EPS)
            std = epool.tile([fout, 1], F32, tag="std")
            nc.scalar.activation(out=std[:], in_=var[:], func=AF.Sqrt,
                                 bias=0.0)
            rstd = epool.tile([fout, 1], F32, tag="rstd")
            nc.vector.reciprocal(rstd[:], std[:])
            sc = epool.tile([fout, 1], F32, tag="sc")
            nc.vector.tensor_tensor(out=sc[:], in0=rstd[:],
                                    in1=wsb[f"gamma{l}"][:], op=OP.mult)
            sh1 = epool.tile([fout, 1], F32, tag="sh1")
            nc.vector.tensor_tensor(out=sh1[:], in0=mu[:], in1=sc[:],
                                    op=OP.mult)
            shf = epool.tile([fout, 1], F32, tag="shf")
            nc.vector.tensor_tensor(out=shf[:], in0=wsb[f"beta{l}"][:],
                                    in1=sh1[:], op=OP.subtract)
            nc.scalar.activation(out=y[:], in_=y[:], func=AF.Relu,
                                 scale=sc[:], bias=shf[:])

            if l < 2:
                fo = MID[l]
                x_cur = mlp_chain(
                    [f"nup{l}_0", f"nup{l}_1", f"nup{l}_2", f"nup{l}_3"],
                    [fo, 128, 256, 128, fo],
                    [AF.Relu, AF.Relu, AF.Relu, AF.Relu], [y], "fm")
            else:
                x_cur = [y]

        if KSTAGE >= 6:
            logits = mlp_chain(["out0", "out1", "out2", "out3n"],
                               [MID[2], 128, 256, 128, 1],
                               [AF.Relu, AF.Relu, AF.Relu, AF.Exp],
                               x_cur, "fm",
                               scales=[1.0, 1.0, 1.0, -1.0])
            sig = fmpool.tile([1, FMW], F32, tag="fm")
            nc.vector.tensor_scalar_add(logits[0][:], logits[0][:], 1.0)
            nc.vector.reciprocal(sig[:], logits[0][:])
            nc.sync.dma_start(out=out_ext[:], in_=sig[:, 0:NPC])
        else:
            nc.sync.dma_start(out=out_ext[:], in_=x_cur[0][0:1, 0:NPC])

    nc.compile()
    return nc


# ---------------------------------------------------------------- entry
def _get_compiled(edge_index, enc, gat, bn, nup, out_mlp):
    if "k" not in _CACHE:
        meta = _host_prep(edge_index)
        wts = _pack_weights(enc, gat, bn, nup, out_mlp)
        wshapes = {k: v.shape for k, v in wts.items()}
        nc = _build(meta, wshapes)
        sim = bass_interp.MultiCoreSim(nc, NCORES, require_finite=False, require_nnan=False)
        _CACHE["k"] = (meta, wts, nc, sim)
    return _CACHE["k"]


def _load_inputs(sim, meta, wts, x):
    for c in range(NCORES):
        t = sim.cores[c]
        t.tensor("x_in")[:] = np.ascontiguousarray(x[meta["node_of"][c]].T)
        t.tensor("idx128")[:] = meta["idxt"][128][c]
        t.tensor("idx256")[:] = meta["idxt"][256][c]
        for name, arr in wts.items():
            t.tensor(name)[:] = arr


def kernel(x, edge_index, enc, gat, bn, nup, out_mlp):
    import time

    x = np.asarray(x, np.float32)
    meta, wts, nc, sim = _get_compiled(edge_index, enc, gat, bn, nup, out_mlp)
    _load_inputs(sim, meta, wts, x)

    t0 = time.time()
    res = sim.run_on_hw_raw()
    kernel._last_wall_s = time.time() - t0

    out = np.empty((N_NODES, 1), np.float32)
    for c in range(NCORES):
        out[meta["node_of"][c], 0] = res.results[c]["out"][0]
    return out
